# revision 6
# baseline (speedup 1.0000x reference)
"""Trainium2 Bass kernel for nn_CLIP_69458211111620 (v2: fused pipeline).

Data-parallel over batch B=128 across 8 NeuronCores (16 batches/core).
Single fused pass per batch (no DRAM staging), software-pipelined 4 deep:
  P1(i): emb matmuls + LN + transpose -> S
  P2(i-1): RT/ab/sc/V/scoresT matmuls, exp (pre-transposed softmax), fused, LN
  P3(i-2): fNT transpose, pooling MLP hT
  P4(i-3): pool softmax + pooled columns
then fc head on SBUF-prefetched weights.

Precision: fp8 e4m3 + DoubleRow for big matmuls where the error budget
allows (sim.py ablations), bf16 elsewhere, f32r where critical.
"""
import sys

sys.path.insert(0, "/opt/trn_rl_repo")

import numpy as np
import ml_dtypes

NCORES = 8
NB = 16          # batches per core
T, C, D, DF, H = 1024, 512, 1024, 2048, 64
ISD = 1.0 / 32.0  # 1/sqrt(D)
EPS = 1e-5

# ---- precision config (validated by sim.py ablations) -----------------
# fp8 e4m3 on any attention-chain operand exceeds the 2e-2 budget (the
# softmax sharply amplifies quantization noise); all-bf16 sims at 6.6e-3.
EMB_FP8 = False   # x, wemb e4m3 (DoubleRow)
ATTN_FP8 = False  # S, M, wqb e4m3 -> RT/ab/sc DoubleRow
V_FP8 = False     # wv e4m3 (uses e4m3 S) -> V matmul DoubleRow
ABSC_DT = "bf16"  # scores matmul operand dtype ("bf16" | "f32r")
HEAD_DT = "bf16"  # fc head weight dtype ("bf16" | "f32r")

S_EMB = 64.0 if EMB_FP8 else 1.0
S_M = 16.0 if ATTN_FP8 else 1.0
S_WQB = 16.0 if ATTN_FP8 else 1.0
S_WV = 32.0 if V_FP8 else 1.0

E4NP = ml_dtypes.float8_e4m3
BFNP = ml_dtypes.bfloat16


def _round_f32r(x):
    u = np.ascontiguousarray(x, dtype=np.float32).view(np.uint32).copy()
    lsb = (u >> np.uint32(12)) & np.uint32(1)
    u += np.uint32(0x7FF) + lsb
    u &= np.uint32(0xFFFFF000)
    return u.view(np.float32)


def _chunk_major(v, nchunk):
    return np.ascontiguousarray(
        np.asarray(v, dtype=np.float32).reshape(nchunk, 128).T
    )


def _sbuf_layout(w, nk):
    """[nk*128, F] -> [128, nk, F] partition-major image."""
    w = np.asarray(w, dtype=np.float32)
    f = w.shape[1]
    return np.ascontiguousarray(
        w.reshape(nk, 128, f).transpose(1, 0, 2)
    )


def _cast(x, dt):
    if dt == "e4m3":
        return np.clip(x, -240.0, 240.0).astype(E4NP)
    if dt == "bf16":
        return np.asarray(x, np.float32).astype(BFNP)
    return _round_f32r(x)


def _build(age_scale_f, bf3_f, bemb_nz, bv_nz, sim_acts=False):
    import concourse.tile as tile
    import concourse.bass as bass
    from concourse import bacc, mybir

    F32 = mybir.dt.float32
    F32R = mybir.dt.float32r
    BF16 = mybir.dt.bfloat16
    E4M3 = mybir.dt.float8e4
    AF = mybir.ActivationFunctionType
    ALU = mybir.AluOpType
    AX = mybir.AxisListType
    PM = mybir.MatmulPerfMode
    ts = bass.ts
    AF_LRELU = AF.Relu if sim_acts else AF.Lrelu

    I32 = mybir.dt.int32
    RSQRT_MAGIC = 0x5F3759DF

    SDT = E4M3 if (ATTN_FP8 or V_FP8) else BF16
    XDT = E4M3 if EMB_FP8 else BF16
    WEDT = E4M3 if EMB_FP8 else BF16
    MDT = E4M3 if ATTN_FP8 else BF16
    WVDT = E4M3 if V_FP8 else BF16
    ABDT = F32R if ABSC_DT == "f32r" else BF16
    HDT = F32R if HEAD_DT == "f32r" else BF16

    def kch(n, fp8):
        """Chunk iteration: DoubleRow pairs if fp8 else single chunks."""
        if fp8:
            return [
                (slice(2 * p, 2 * p + 2), p == 0, p == n // 2 - 1, PM.DoubleRow)
                for p in range(n // 2)
            ]
        return [(slice(k, k + 1), k == 0, k == n - 1, None) for k in range(n)]

    nc = bacc.Bacc("TRN2", target_bir_lowering=False, debug=False)

    def inp(name, shape, dt):
        return nc.dram_tensor(name, shape, dt, kind="ExternalInput").ap()

    X = inp("x", (NB, 128, 8, C), XDT)
    WEMB = inp("wemb", (128, 8, D), WEDT)
    MQK = inp("m_mat", (128, 8, D), MDT)
    WQB = inp("wqb", (128, 8, C), MDT)
    WV = inp("wv", (128, 8, D), WVDT)
    WP1G = inp("wp1g", (128, 8, 128), BF16)
    WP2 = inp("wp2", (H, 1), BF16)
    WF1 = inp("wf1", (32, 128, 512), HDT)
    WF2 = inp("wf2", (64, 128, 512), HDT)
    WF3C = inp("wf3c", (128, DF // 128, 2), F32R)
    GS = inp("gs_c", (128, 8), F32)
    BS = inp("bs_c", (128, 8), F32)
    GF = inp("gf_c", (128, 8), F32)
    BF_ = inp("bf_c", (128, 8), F32)
    BP1E = inp("bp1e", (H, 1), F32)
    BF1 = inp("bf1_c", (128, 16), F32)
    BF2 = inp("bf2_c", (128, 16), F32)
    IDENT = inp("ident", (128, 128), BF16)
    BEMB = inp("bemb_row", (1, D), F32R) if bemb_nz else None
    BVR = inp("bv_row", (1, D), F32R) if bv_nz else None
    RUL = nc.dram_tensor("rul", (NB, 1), F32, kind="ExternalOutput").ap()

    with tile.TileContext(nc) as tc:
        # ---- long-lived tiles ----------------------------------------
        glob = tc.alloc_tile_pool(name="glob", bufs=1)
        id_sb = glob.tile([128, 128], BF16, name="id_sb")
        magic_t = glob.tile([128, 4], I32, name="magic_t")
        ages_t = glob.tile([128, 1], F32, name="ages_t")
        ones_c = glob.tile([128, 1], BF16, name="ones_c")
        idf1 = glob.tile([1, 1], F32, name="idf1")
        pooledT = glob.tile([128, 8, NB], BF16, name="pooledT")
        gs_sb = glob.tile([128, 8], F32, name="gs_sb")
        bs_sb = glob.tile([128, 8], F32, name="bs_sb")
        gf_sb = glob.tile([128, 8], F32, name="gf_sb")
        bf_sb = glob.tile([128, 8], F32, name="bf_sb")
        nc.sync.dma_start(id_sb[:], IDENT[:])
        nc.sync.dma_start(gs_sb[:], GS[:])
        nc.sync.dma_start(bs_sb[:], BS[:])
        nc.sync.dma_start(gf_sb[:], GF[:])
        nc.sync.dma_start(bf_sb[:], BF_[:])
        nc.gpsimd.memset(magic_t[:], RSQRT_MAGIC)
        nc.gpsimd.memset(ages_t[:], age_scale_f)
        nc.gpsimd.memset(ones_c[:], 1.0)
        nc.gpsimd.memset(idf1[:], 1.0)
        ones_r = None
        if bemb_nz or bv_nz:
            ones_r = glob.tile([1, 128], F32R, name="ones_r")
            nc.gpsimd.memset(ones_r[:], 1.0)
        bemb_sb = None
        if bemb_nz:
            bemb_sb = glob.tile([1, D], F32R, name="bemb_sb")
            nc.sync.dma_start(bemb_sb[:], BEMB[:])
        bv_sb = None
        if bv_nz:
            bv_sb = glob.tile([1, D], F32R, name="bv_sb")
            nc.sync.dma_start(bv_sb[:], BVR[:])

        # ---- weights --------------------------------------------------
        wts = tc.alloc_tile_pool(name="wts", bufs=1)
        wemb_sb = wts.tile([128, 8, D], WEDT, name="wemb_sb")
        m_sb = wts.tile([128, 8, D], MDT, name="m_sb")
        wqb_sb = wts.tile([128, 8, C], MDT, name="wqb_sb")
        wv_sb = wts.tile([128, 8, D], WVDT, name="wv_sb")
        wp1_sb = wts.tile([128, 8, 128], BF16, name="wp1_sb")
        wp2_sb = wts.tile([H, 1], BF16, name="wp2_sb")
        bp1_sb = wts.tile([H, 1], F32, name="bp1_sb")
        bf1_sb = wts.tile([128, 16], F32, name="bf1_sb")
        bf2_sb = wts.tile([128, 16], F32, name="bf2_sb")
        wf3_sb = wts.tile([128, 16, 2], F32R, name="wf3_sb")
        weight_dmas = [
            (m_sb, MQK), (wqb_sb, WQB), (wv_sb, WV), (wp1_sb, WP1G),
            (wp2_sb, WP2), (bp1_sb, BP1E), (bf1_sb, BF1), (bf2_sb, BF2),
            (wf3_sb, WF3C),
        ]

        def emit_rsqrt(pool, v_ap, w, tagp, eps, iters=2):
            """[128,w] -> 1/sqrt(v + eps) elementwise on DVE (Quake+Newton)."""
            ve = pool.tile([128, w], F32, name=f"{tagp}ve", tag=f"{tagp}ve")
            nc.vector.tensor_scalar(ve[:], v_ap, eps, None, op0=ALU.add)
            y = pool.tile([128, w], F32, name=f"{tagp}y0", tag=f"{tagp}y0")
            nc.vector.tensor_scalar(
                y.bitcast(I32)[:], ve.bitcast(I32)[:], 1, None,
                op0=ALU.logical_shift_right,
            )
            nc.vector.scalar_tensor_tensor(
                y.bitcast(I32)[:], y.bitcast(I32)[:], -1, magic_t[:, 0:w],
                op0=ALU.mult, op1=ALU.add,
            )
            for it in range(iters):
                a = pool.tile([128, w], F32, name=f"{tagp}a{it}", tag=f"{tagp}a{it}")
                nc.vector.tensor_tensor(a[:], y[:], y[:], op=ALU.mult)
                nc.vector.tensor_tensor(a[:], a[:], ve[:], op=ALU.mult)
                nc.vector.tensor_scalar(
                    a[:], a[:], -0.5, 1.5, op0=ALU.mult, op1=ALU.add
                )
                nc.vector.tensor_tensor(y[:], y[:], a[:], op=ALU.mult)
            return y

        # ---- pipelined main loop -------------------------------------
        with (
            tc.tile_pool(name="px", bufs=2) as px,
            tc.tile_pool(name="pw3", bufs=16) as pw3,
            tc.tile_pool(name="psen", bufs=1) as psen,
            tc.tile_pool(name="pS", bufs=2) as pS,
            tc.tile_pool(name="pmid", bufs=1) as pmid,
            tc.tile_pool(name="pfn", bufs=3) as pfn,
            tc.tile_pool(name="psc1", bufs=2) as psc1,
            tc.tile_pool(name="psc2", bufs=2) as psc2,
            tc.tile_pool(name="pgel", bufs=1) as pgel,
            tc.tile_pool(name="ps_emb", bufs=3, space="PSUM") as ps_emb,
            tc.tile_pool(name="ps_main", bufs=3, space="PSUM") as ps_main,
            tc.tile_pool(name="ps_small", bufs=2, space="PSUM") as ps_small,
        ):
            st = [dict() for _ in range(NB)]

            def p1_emb(i):
                s = st[i]
                if i == 0:
                    xb = px.tile([128, 8, C], XDT, name="xb", tag="xb")
                    nc.sync.dma_start(xb[:], X[0])
                    nc.sync.dma_start(wemb_sb[:], WEMB[:])
                    for w_t, w_d in weight_dmas:
                        nc.sync.dma_start(w_t[:], w_d[:])
                    s["xb"] = xb
                xb = s.pop("xb")
                if i + 1 < NB:
                    xb2 = px.tile([128, 8, C], XDT, name="xb2", tag="xb")
                    nc.sync.dma_start(xb2[:], X[i + 1])
                    st[i + 1]["xb"] = xb2
                sen_n = psen.tile([128, 4, D], BF16, name="sen_n", tag="sen")
                for ck in range(4):
                    bn6 = psc1.tile([128, 2, 6], F32, name="bn6", tag="st6")
                    ph2 = []
                    for dh in range(2):
                        ps_s = ps_emb.tile([128, 512], F32, name="ps_s", tag="ps_s")
                        for sl, sta, stp, pm in kch(8, EMB_FP8):
                            nc.tensor.matmul(
                                ps_s[:],
                                xb[:, sl, ts(ck, 128)],
                                wemb_sb[:, sl, dh * 512:(dh + 1) * 512],
                                start=sta,
                                stop=(stp and not bemb_nz),
                                perf_mode=pm,
                            )
                        if bemb_nz:
                            nc.tensor.matmul(
                                ps_s[:],
                                ones_r[0:1, :],
                                bemb_sb[0:1, dh * 512:(dh + 1) * 512],
                                start=False, stop=True,
                            )
                        nc.vector.bn_stats(bn6[:, dh, :], ps_s[:])
                        ph2.append(ps_s)
                    bnag = psc1.tile([128, 2], F32, name="bnag", tag="bnag")
                    nc.vector.bn_aggr(bnag[:], bn6[:])
                    i_t = emit_rsqrt(
                        psc1, bnag[:, 1:2], 1, "l1", EPS * S_EMB * S_EMB
                    )
                    negmi = psc1.tile([128, 1], F32, name="negmi", tag="negmi")
                    nc.vector.scalar_tensor_tensor(
                        negmi[:], bnag[:, 0:1], -1.0, i_t[:],
                        op0=ALU.mult, op1=ALU.mult,
                    )
                    for dh in range(2):
                        nc.scalar.activation(
                            sen_n[:, ck, dh * 512:(dh + 1) * 512],
                            ph2[dh][:], AF.Identity,
                            bias=negmi[:], scale=i_t[:],
                        )
                s["sen_n"] = sen_n

            def p1_tr(i):
                s = st[i]
                sen_n = s.pop("sen_n")
                S_t = pS.tile([128, 8, C], SDT, name="S_t", tag="S")
                for dk in range(8):
                    ps_t = ps_small.tile([128, 512], BF16, name="ps_t", tag="sm")
                    for ck in range(4):
                        nc.tensor.transpose(
                            ps_t[:, ts(ck, 128)], sen_n[:, ck, ts(dk, 128)],
                            id_sb[:],
                        )
                    nc.scalar.activation(
                        S_t[:, dk, :], ps_t[:], AF.Identity,
                        bias=bs_sb[:, dk:dk + 1], scale=gs_sb[:, dk:dk + 1],
                    )
                s["S"] = S_t

            def p2_big(i):
                s = st[i]
                S_t = s.pop("S")
                # RT = (S M)^T  [e(8), n=C]
                RT = pmid.tile([128, 8, C], MDT, name="RT", tag="RT")
                for ec in range(8):
                    ptr = ps_main.tile([128, C], F32, name="ptr", tag="pm")
                    for sl, sta, stp, pm in kch(8, ATTN_FP8):
                        nc.tensor.matmul(
                            ptr[:], m_sb[:, sl, ts(ec, 128)], S_t[:, sl, :],
                            start=sta, stop=stp, perf_mode=pm,
                        )
                    nc.scalar.activation(RT[:, ec, :], ptr[:], AF.Copy)
                # ab = S Wqb * isd/s_wqb  [n(4), m=C]
                ab = pmid.tile([128, 4, C], ABDT, name="ab", tag="ab")
                for nk in range(4):
                    pa = ps_main.tile([128, C], F32, name="pa", tag="pm")
                    for sl, sta, stp, pm in kch(8, ATTN_FP8):
                        nc.tensor.matmul(
                            pa[:], S_t[:, sl, ts(nk, 128)], wqb_sb[:, sl, :],
                            start=sta, stop=stp, perf_mode=pm,
                        )
                    nc.scalar.activation(
                        ab[:, nk, :], pa[:], AF.Copy, scale=float(ISD / S_WQB)
                    )
                # sc = R S^T * isd/s_M + age  [n(4), m=C]
                sc = pmid.tile([128, 4, C], ABDT, name="sc", tag="sc")
                for nk in range(4):
                    pa = ps_main.tile([128, C], F32, name="pa2", tag="pm")
                    for sl, sta, stp, pm in kch(8, ATTN_FP8):
                        nc.tensor.matmul(
                            pa[:], RT[:, sl, ts(nk, 128)], S_t[:, sl, :],
                            start=sta, stop=stp, perf_mode=pm,
                        )
                    nc.scalar.activation(
                        sc[:, nk, :], pa[:], AF.Identity,
                        bias=ages_t[:], scale=float(ISD / S_M),
                    )
                # V = S Wv  [m(4), D]
                V = pmid.tile([128, 4, D], BF16, name="V", tag="V")
                for mk in range(4):
                    for dh in range(2):
                        pv = ps_main.tile([128, 512], F32, name="pv", tag="pm")
                        for sl, sta, stp, pm in kch(8, V_FP8):
                            nc.tensor.matmul(
                                pv[:],
                                S_t[:, sl, ts(mk, 128)],
                                wv_sb[:, sl, dh * 512:(dh + 1) * 512],
                                start=sta,
                                stop=(stp and not bv_nz),
                                perf_mode=pm,
                            )
                        if bv_nz:
                            nc.tensor.matmul(
                                pv[:],
                                ones_r[0:1, :],
                                bv_sb[0:1, dh * 512:(dh + 1) * 512],
                                start=False, stop=True,
                            )
                        nc.vector.tensor_copy(
                            V[:, mk, dh * 512:(dh + 1) * 512], pv[:]
                        )
                s["V"] = V
                # scoresT[k,n] = sum_j sc[j,k] ab[j,n]; exp -> expT (bf16)
                expT = pmid.tile([128, 4, C], BF16, name="expT", tag="expT")
                for kk in range(4):
                    psc = ps_main.tile([128, C], F32, name="psc", tag="pm")
                    for jk in range(4):
                        nc.tensor.matmul(
                            psc[:], sc[:, jk, ts(kk, 128)], ab[:, jk, :],
                            start=(jk == 0), stop=(jk == 3),
                        )
                    nc.scalar.activation(expT[:, kk, :], psc[:], AF.Exp)
                s["expT"] = expT

            def p2_fused(i):
                s = st[i]
                V = s.pop("V")
                expT = s.pop("expT")
                # row sums of exp (over k) as columns per nk + reciprocal
                pssum = ps_small.tile([128, 4], F32, name="pssum", tag="sm")
                for nk in range(4):
                    for kk in range(4):
                        nc.tensor.matmul(
                            pssum[:, nk:nk + 1],
                            expT[:, kk, ts(nk, 128)],
                            ones_c[:],
                            start=(kk == 0), stop=(kk == 3),
                        )
                recips = psc2.tile([128, 4], F32, name="recips", tag="rec")
                nc.vector.reciprocal(recips[:], pssum[:])
                # fused = softmax @ V * isd (LN folded)
                fN = pfn.tile([128, 4, D], BF16, name="fN", tag="fN")
                bn6f = psc2.tile([128, 2, 6], F32, name="bn6f", tag="bn6f")
                bnagf = psc2.tile([128, 2], F32, name="bnagf", tag="bnagf")
                for nk in range(4):
                    pfs = []
                    for dh in range(2):
                        pf = ps_main.tile([128, 512], F32, name="pf", tag="pm")
                        for kk in range(4):
                            nc.tensor.matmul(
                                pf[:],
                                expT[:, kk, ts(nk, 128)],
                                V[:, kk, dh * 512:(dh + 1) * 512],
                                start=(kk == 0), stop=(kk == 3),
                            )
                        nc.vector.bn_stats(bn6f[:, dh, :], pf[:])
                        pfs.append(pf)
                    nc.vector.bn_aggr(bnagf[:], bn6f[:])
                    s_t = psc2.tile([128, 1], F32, name="s_t", tag="s_t")
                    nc.vector.tensor_scalar(
                        s_t[:], recips[:, nk:nk + 1], float(ISD / S_WV), None,
                        op0=ALU.mult,
                    )
                    s2_t = psc2.tile([128, 1], F32, name="s2_t", tag="s2_t")
                    nc.vector.tensor_tensor(s2_t[:], s_t[:], s_t[:], op=ALU.mult)
                    vs_t = psc2.tile([128, 1], F32, name="vs_t", tag="vs_t")
                    nc.vector.scalar_tensor_tensor(
                        vs_t[:], bnagf[:, 1:2], 1.0, s2_t[:],
                        op0=ALU.mult, op1=ALU.mult,
                    )
                    i2_t = emit_rsqrt(psc2, vs_t[:], 1, "l2", EPS)
                    se_t = psc2.tile([128, 1], F32, name="se_t", tag="se_t")
                    nc.vector.tensor_tensor(se_t[:], s_t[:], i2_t[:], op=ALU.mult)
                    be_t = psc2.tile([128, 1], F32, name="be_t", tag="be_t")
                    nc.vector.scalar_tensor_tensor(
                        be_t[:], bnagf[:, 0:1], -1.0, se_t[:],
                        op0=ALU.mult, op1=ALU.mult,
                    )
                    for dh in range(2):
                        nc.scalar.activation(
                            fN[:, nk, dh * 512:(dh + 1) * 512], pfs[dh][:],
                            AF.Identity, bias=be_t[:], scale=se_t[:],
                        )
                s["fN"] = fN

            def p3(i):
                s = st[i]
                fN = s["fN"]
                fNT = pmid.tile([128, 8, C], BF16, name="fNT", tag="fNT")
                for dk in range(8):
                    ptf = ps_small.tile([128, 512], BF16, name="ptf", tag="sm")
                    for nkk in range(4):
                        nc.tensor.transpose(
                            ptf[:, ts(nkk, 128)], fN[:, nkk, ts(dk, 128)],
                            id_sb[:],
                        )
                    nc.vector.tensor_copy(fNT[:, dk, :], ptf[:])
                ph = ps_emb.tile([128, C], F32, name="ph", tag="ps_s")
                for kc in range(8):
                    nc.tensor.matmul(
                        ph[:], wp1_sb[:, kc, :], fNT[:, kc, :],
                        start=(kc == 0), stop=(kc == 7),
                    )
                # gelu (tanh formula; Square/Tanh share the Exp table set)
                gx = pgel.tile([H, C], F32, name="gx", tag="gx")
                nc.scalar.activation(gx[:], ph[0:H, :], AF.Identity, bias=bp1_sb[:])
                g2 = pgel.tile([H, C], F32, name="g2", tag="g2")
                nc.scalar.activation(g2[:], gx[:], AF.Square)
                nc.vector.tensor_scalar(
                    g2[:], g2[:], 0.044715 * 0.7978845608028654,
                    0.7978845608028654, op0=ALU.mult, op1=ALU.add,
                )
                nc.vector.tensor_tensor(g2[:], g2[:], gx[:], op=ALU.mult)
                nc.scalar.activation(g2[:], g2[:], AF.Tanh)
                nc.vector.tensor_scalar(g2[:], g2[:], 1.0, None, op0=ALU.add)
                hT = pgel.tile([H, C], BF16, name="hT", tag="hT")
                nc.vector.scalar_tensor_tensor(
                    hT[:], g2[:], 0.5, gx[:], op0=ALU.mult, op1=ALU.mult,
                )
                s["hT"] = hT

            def p4a(i):
                s = st[i]
                hT = s.pop("hT")
                pps = ps_emb.tile([1, C], F32, name="pps", tag="ps_s")
                nc.tensor.matmul(pps[:], wp2_sb[:], hT[:], start=True, stop=True)
                pnm = psc2.tile([1, 1], F32, name="pnm", tag="pnm")
                nc.vector.tensor_reduce(
                    pnm[:], pps[:], axis=AX.X, op=ALU.max, negate=True
                )
                pw = pgel.tile([1, C], BF16, name="pw", tag="row")
                pse = psc2.tile([1, 1], F32, name="pse", tag="pse")
                nc.scalar.activation(
                    pw[:], pps[:], AF.Exp, bias=pnm[:], accum_out=pse[:]
                )
                prc = psc2.tile([1, 1], F32, name="prc", tag="prc")
                nc.vector.reciprocal(prc[:], pse[:])
                pwn = pgel.tile([1, C], F32, name="pwn", tag="rown")
                nc.vector.tensor_scalar(
                    pwn[:], pw[:], prc[0:1, 0:1], None, op0=ALU.mult
                )
                s["pwn"] = pwn

            def p4b(i):
                s = st[i]
                pwn = s.pop("pwn")
                fN = s.pop("fN")
                ppw = ps_small.tile([128, 4], F32, name="ppw", tag="sm")
                for nk in range(4):
                    nc.tensor.transpose(
                        ppw[:, nk:nk + 1], pwn[0:1, ts(nk, 128)], idf1[:]
                    )
                pwc = pgel.tile([128, 4, 2], BF16, name="pwc", tag="pwc")
                nc.scalar.activation(pwc[:, :, 0], ppw[:, 0:4], AF.Copy)
                nc.scalar.activation(pwc[:, :, 1], ppw[:, 0:4], AF.Copy)
                for dk in range(8):
                    pp = ps_small.tile([128, 2], F32, name="pp", tag="sm")
                    for nk in range(4):
                        nc.tensor.matmul(
                            pp[:], fN[:, nk, ts(dk, 128)], pwc[:, nk, :],
                            start=(nk == 0), stop=(nk == 3),
                        )
                    nc.scalar.activation(
                        pooledT[:, dk, i:i + 1], pp[:, 0:1], AF.Identity,
                        bias=bf_sb[:, dk:dk + 1], scale=gf_sb[:, dk:dk + 1],
                    )

            for i in range(NB + 3):
                if i < NB:
                    p1_emb(i)
                if 1 <= i <= NB:
                    p2_big(i - 1)
                if 3 <= i:
                    p4a(i - 3)
                if 1 <= i <= NB:
                    p2_fused(i - 1)
                if 3 <= i:
                    p4b(i - 3)
                if i < NB:
                    p1_tr(i)
                if 2 <= i <= NB + 1:
                    p3(i - 2)

            # =================== fc head ==============================
            h1T = pmid.tile([128, 16, NB], BF16, name="h1T", tag="RT")
            h2T = pmid.tile([128, 16, NB], F32R, name="h2T", tag="fNT")
            for g in range(4):
                pg = ps_main.tile([128, 4, NB], F32, name=f"pg{g}", tag="pm")
                wts1 = []
                for kd in range(8):
                    wt = pw3.tile([128, 512], HDT, name="wt1", tag="w3")
                    nc.sync.dma_start(wt[:], WF1[g * 8 + kd])
                    wts1.append(wt)
                for j in range(4):
                    for kd in range(8):
                        nc.tensor.matmul(
                            pg[:, j, :], wts1[kd][:, ts(j, 128)],
                            pooledT[:, kd, :],
                            start=(kd == 0), stop=(kd == 7),
                        )
                for j in range(4):
                    mf = g * 4 + j
                    nc.scalar.activation(
                        h1T[:, mf, :], pg[:, j, :], AF_LRELU,
                        bias=bf1_sb[:, mf:mf + 1], alpha=0.01,
                    )

            for g in range(4):
                pg = ps_main.tile([128, 4, NB], F32, name=f"qg{g}", tag="pm")
                wts2 = []
                for kf in range(16):
                    wt = pw3.tile([128, 512], HDT, name="wt2", tag="w3")
                    nc.sync.dma_start(wt[:], WF2[g * 16 + kf])
                    wts2.append(wt)
                for j in range(4):
                    for kf in range(16):
                        nc.tensor.matmul(
                            pg[:, j, :], wts2[kf][:, ts(j, 128)], h1T[:, kf, :],
                            start=(kf == 0), stop=(kf == 15),
                        )
                for j in range(4):
                    mf = g * 4 + j
                    nc.scalar.activation(
                        h2T[:, mf, :], pg[:, j, :], AF.Identity,
                        bias=bf2_sb[:, mf:mf + 1],
                    )

            prul = ps_small.tile([NB, 2], F32, name="prul", tag="sm")
            for k in range(16):
                nc.tensor.matmul(
                    prul[:], h2T[:, k, :], wf3_sb[:, k, :],
                    start=(k == 0), stop=(k == 15),
                )
            bf3_t = psc2.tile([NB, 1], F32, name="bf3_t", tag="bf3")
            nc.gpsimd.memset(bf3_t[:], bf3_f)
            rul_sb = psc2.tile([NB, 1], F32, name="rul_sb", tag="rul")
            nc.scalar.activation(rul_sb[:], prul[:, 0:1], AF.Abs, bias=bf3_t[:])
            nc.sync.dma_start(RUL[:], rul_sb[:])

        wts.release()
        glob.release()

    nc.compile()
    return nc


def _prep_in_maps(inputs):
    f32 = np.float32
    x_enc = np.asarray(inputs["x_enc"], f32)
    W_emb = np.asarray(inputs["W_emb"], f32)
    b_emb = np.asarray(inputs["b_emb"], f32)
    g_s = np.asarray(inputs["g_s"], f32)
    b_s = np.asarray(inputs["b_s"], f32)
    basis = np.asarray(inputs["basis"], np.float64)
    Wq = np.asarray(inputs["Wq"], np.float64)
    bq = np.asarray(inputs["bq"], f32)
    Wk = np.asarray(inputs["Wk"], np.float64)
    bk = np.asarray(inputs["bk"], f32)
    Wv = np.asarray(inputs["Wv"], f32)
    bv = np.asarray(inputs["bv"], f32)
    g_f = np.asarray(inputs["g_f"], f32)
    b_f = np.asarray(inputs["b_f"], f32)
    Wp1 = np.asarray(inputs["Wp1"], f32)
    bp1 = np.asarray(inputs["bp1"], f32)
    Wp2 = np.asarray(inputs["Wp2"], f32)
    Wf1 = np.asarray(inputs["Wf1"], f32)
    bf1 = np.asarray(inputs["bf1"], f32)
    Wf2 = np.asarray(inputs["Wf2"], f32)
    bf2 = np.asarray(inputs["bf2"], f32)
    Wf3 = np.asarray(inputs["Wf3"], f32)

    assert not (np.any(bq) or np.any(bk)), "folded QK path requires bq=bk=0"

    M = (Wq @ Wk.T).astype(f32)
    Wqb = (Wq @ basis.T).astype(f32)

    wp1g = np.zeros((D, 128), f32)
    wp1g[:, :H] = g_f[:, None] * Wp1
    bp1e = (b_f @ Wp1 + bp1).reshape(H, 1).astype(f32)

    e_dt = "e4m3" if EMB_FP8 else "bf16"
    m_dt = "e4m3" if ATTN_FP8 else "bf16"
    v_dt = "e4m3" if V_FP8 else "bf16"
    h_dt = "bf16" if HEAD_DT == "bf16" else "f32r"

    common = {
        "wemb": _cast(_sbuf_layout(W_emb * S_EMB, 8), e_dt),
        "m_mat": _cast(_sbuf_layout(M * S_M, 8), m_dt),
        "wqb": _cast(_sbuf_layout(Wqb * S_WQB, 8), m_dt),
        "wv": _cast(_sbuf_layout(Wv * S_WV, 8), v_dt),
        "wp1g": _cast(_sbuf_layout(wp1g, 8), "bf16"),
        "wp2": Wp2.astype(BFNP),
        "wf1": _cast(
            Wf1.reshape(8, 128, 4, 512).transpose(2, 0, 1, 3).reshape(32, 128, 512),
            h_dt,
        ),
        "wf2": _cast(
            Wf2.reshape(16, 128, 4, 512).transpose(2, 0, 1, 3).reshape(64, 128, 512),
            h_dt,
        ),
        "wf3c": _round_f32r(
            np.repeat(_chunk_major(Wf3[:, 0], 16)[:, :, None], 2, axis=2)
        ),
        "gs_c": _chunk_major(g_s, 8),
        "bs_c": _chunk_major(b_s, 8),
        "gf_c": _chunk_major(g_f, 8),
        "bf_c": _chunk_major(b_f, 8),
        "bp1e": bp1e,
        "bf1_c": _chunk_major(bf1, 16),
        "bf2_c": _chunk_major(bf2, 16),
        "ident": np.eye(128).astype(BFNP),
    }
    bemb_nz = bool(np.any(b_emb))
    bv_nz = bool(np.any(bv))
    if bemb_nz:
        common["bemb_row"] = _round_f32r(b_emb.reshape(1, D) * S_EMB)
    if bv_nz:
        common["bv_row"] = _round_f32r(bv.reshape(1, D) * S_WV)

    in_maps = []
    for c in range(NCORES):
        m = dict(common)
        xs = x_enc[c * NB:(c + 1) * NB]
        m["x"] = _cast(
            xs.reshape(NB, 8, 128, C).transpose(0, 2, 1, 3), e_dt
        )
        in_maps.append(m)

    age_scale_f = float(np.asarray(inputs["age_scale"], f32))
    bf3_f = float(np.asarray(inputs["bf3"], f32).reshape(-1)[0])
    return in_maps, age_scale_f, bf3_f, bemb_nz, bv_nz


_NC_CACHE = {}


def build_program(inputs, sim_acts=False):
    in_maps, age_scale_f, bf3_f, bemb_nz, bv_nz = _prep_in_maps(inputs)
    key = (age_scale_f, bf3_f, bemb_nz, bv_nz, sim_acts)
    if key not in _NC_CACHE:
        _NC_CACHE[key] = _build(age_scale_f, bf3_f, bemb_nz, bv_nz, sim_acts)
    return _NC_CACHE[key], in_maps


def kernel(**inputs):
    from concourse.bass_utils import run_bass_kernel_spmd

    nc, in_maps = build_program(inputs)
    res = run_bass_kernel_spmd(nc, in_maps, core_ids=list(range(NCORES)))
    out = np.concatenate(
        [res.results[c]["rul"] for c in range(NCORES)], axis=0
    ).astype(np.float32)
    return out
